# revision 23
# baseline (speedup 1.0000x reference)
"""Trainium2 Bass kernel: 8-expert top-2 MoE layer (SwiGLU experts).

Sharding: paired expert parallelism across 8 NeuronCores. Experts are
paired heaviest-with-lightest; each pair lands on two cores that both
process BOTH experts' full token sets over HALF of the intermediate dim
(an exact decomposition: gate/up split along their output dim, the down
projection's partial contractions summed on the host). Per-core work is
(max heavy load + max light load)/2 token-equivalents instead of the max
expert load. The host performs the router (exact fp64 softmax/top-2,
shipped as per-token combine weights) and the token dispatch/combine.
The FFN runs in bf16 with fp32 PSUM accumulation.

Self-contained: hardcodes all shapes from the problem spec.
"""

import os

import numpy as np

# Problem constants
H = 1024  # hidden dim
I = 4096  # intermediate dim
E = 8  # experts
P = 128  # SBUF partitions
IH = I // 2  # intermediate features per core (half of I)

# Tiling constants
TB = 512  # tokens per block (matmul moving free dim)
IS = 1024  # intermediate features resident per weight wave
N_SUPER = IH // IS  # weight waves per segment (= 2)
IT = IS // P  # i-tiles per wave
HO = H // P  # h chunks (contraction tiles)
HH = H // 512  # output column halves for the down projection
NQ = IT // 2  # quarter sub-tiles for the first wave's weight load


def _blocks(Tc):
    """Token blocks, largest first: the first wave's first block consumes
    the just-arriving weights at the slowest rate, and the smallest block
    lands last so each wave's flush is minimal."""
    assert Tc % P == 0 and Tc >= 256
    sizes = []
    rem = Tc
    while rem > 767:
        sizes.append(TB)
        rem -= TB
    if rem > 512:
        sizes.extend([rem - 256, 256])
    elif rem:
        sizes.append(rem)
    sizes.sort(reverse=True)
    blocks = []
    t = 0
    for tb in sizes:
        blocks.append((t, tb))
        t += tb
    return blocks


def build_moe(TCA: int, TCB: int):
    """Per-core program: segments A/B of TCA/TCB tokens, half-I each."""
    import concourse.bass as bass  # noqa: F401
    import concourse.mybir as mybir
    import concourse.tile as tile
    from concourse import bacc

    f32 = mybir.dt.float32
    bf16 = mybir.dt.bfloat16
    Alu = mybir.AluOpType
    Act = mybir.ActivationFunctionType

    nc = bacc.Bacc(
        "TRN2", target_bir_lowering=False, debug=False, num_devices=8
    )

    segs = {}
    for s, Tc in (("a", TCA), ("b", TCB)):
        seg = {
            "Tc": Tc,
            "blocks": _blocks(Tc),
            "NW": Tc // P,
            "xT": nc.dram_tensor(f"x{s}", [H, Tc], bf16, kind="ExternalInput").ap(),
            "wg": nc.dram_tensor(f"wg{s}", [H, IH], bf16, kind="ExternalInput").ap(),
            "wu": nc.dram_tensor(f"wu{s}", [H, IH], bf16, kind="ExternalInput").ap(),
            "wd": nc.dram_tensor(f"wd{s}", [IH, H], bf16, kind="ExternalInput").ap(),
            "wal": nc.dram_tensor(f"wal{s}", [P, Tc // P], f32, kind="ExternalInput").ap(),
            # Wave 0 writes f32, wave 1 writes a separate bf16 buffer on
            # the HWDGE queues (no read-modify-write accumulate anywhere;
            # the host adds the two).
            "out": nc.dram_tensor(f"out{s}", [Tc, H], f32, kind="ExternalOutput").ap(),
            "out2": nc.dram_tensor(f"out2{s}", [Tc, H], bf16, kind="ExternalOutput").ap(),
        }
        seg["xT_r"] = seg["xT"].rearrange("(ho p) t -> p ho t", p=P)
        seg["wg_r"] = seg["wg"].rearrange("(ho p) i -> p ho i", p=P)
        seg["wu_r"] = seg["wu"].rearrange("(ho p) i -> p ho i", p=P)
        seg["wd_r"] = seg["wd"].rearrange("(io p) h -> p io h", p=P)
        segs[s] = seg

    with tile.TileContext(nc) as tc:
        with (
            tc.tile_pool(name="singles", bufs=1) as singles,
            tc.tile_pool(name="xres", bufs=1) as xres,
            tc.tile_pool(name="w0", bufs=1) as w0pool,
            tc.tile_pool(name="weights", bufs=2) as wpool,
            tc.tile_pool(name="hp", bufs=2) as hpool,
            tc.tile_pool(name="ep", bufs=3) as epool,
            tc.tile_pool(name="pgu", bufs=2, space="PSUM") as pgu,
            tc.tile_pool(name="pout", bufs=3, space="PSUM") as pout,
        ):
            # x tiles are SHARED between the two segments (union of their
            # block-size multisets) and reloaded at each wave; the WAR
            # dependencies through the tile framework schedule each
            # reload during the preceding wave.
            from collections import Counter

            need = Counter()
            for seg in segs.values():
                c = Counter(tb for _, tb in seg["blocks"])
                for sz, n in c.items():
                    need[sz] = max(need[sz], n)
            xtiles = {
                sz: [
                    xres.tile([P, HO, sz], bf16, tag=f"xt{sz}_{k}", name=f"xt{sz}_{k}")
                    for k in range(n)
                ]
                for sz, n in need.items()
            }
            for s, seg in segs.items():
                seg["wal_sb"] = singles.tile(
                    [P, seg["NW"]], f32, tag=f"wal{s}", name=f"wal{s}"
                )
                used = Counter()
                seg["x_sb"] = []
                for _, tb in seg["blocks"]:
                    seg["x_sb"].append(xtiles[tb][used[tb]])
                    used[tb] += 1

            # Segment A block 0 loads first on the (fast, otherwise idle)
            # gpsimd queue so the PE can start ~17 us in.
            sa = segs["a"]
            t0_0, tb_0 = sa["blocks"][0]
            nc.gpsimd.dma_start(sa["x_sb"][0], sa["xT_r"][:, :, t0_0 : t0_0 + tb_0])

            # Waves: (segment, super) in execution order. Both A-waves
            # run first so A's x loads once; B's x loads exactly once,
            # interleaved into wave a1 as each shared tile is freed.
            waves = [("a", 0), ("a", 1), ("b", 0), ("b", 1)]
            sa_, sb_ = segs["a"], segs["b"]
            a_ids = {id(t): i for i, t in enumerate(sa_["x_sb"])}
            b_load, prologue_b = {}, []
            for bi_, (t0_, tb_) in enumerate(sb_["blocks"]):
                t_ = sb_["x_sb"][bi_]
                src_ = sb_["xT_r"][:, :, t0_ : t0_ + tb_]
                if id(t_) in a_ids:
                    b_load.setdefault(a_ids[id(t_)], []).append((t_, src_))
                else:
                    prologue_b.append((t_, src_))
            for wi, (s, sup) in enumerate(waves):
                seg = segs[s]
                blocks = seg["blocks"]
                x_sb = seg["x_sb"]
                wal_sb = seg["wal_sb"]
                i0 = sup * IS
                if wi == 0:
                    # First wave's gate/up weights race the PE: quarter
                    # tiles split across the queues in measured-rate order
                    # (SWDGE ~260 GB/s, each HWDGE queue ~60 GB/s).
                    wgq, wuq = [], []
                    for q in range(NQ):
                        wgq.append(
                            w0pool.tile([P, HO, 2 * P], bf16, tag=f"wgq{q}", name=f"wgq{q}")
                        )
                        wuq.append(
                            w0pool.tile([P, HO, 2 * P], bf16, tag=f"wuq{q}", name=f"wuq{q}")
                        )
                    for q, eng_g, eng_u in (
                        (0, nc.sync, nc.scalar),
                        (1, nc.gpsimd, nc.gpsimd),
                        (2, nc.sync, nc.scalar),
                        (3, nc.gpsimd, nc.gpsimd),
                    ):
                        c = i0 + q * 2 * P
                        eng_g.dma_start(wgq[q], seg["wg_r"][:, :, c : c + 2 * P])
                        eng_u.dma_start(wuq[q], seg["wu_r"][:, :, c : c + 2 * P])

                    def wgt(it, wgq=wgq):
                        return wgq[it // 2][:, :, (it % 2) * P : (it % 2 + 1) * P]

                    def wut(it, wuq=wuq):
                        return wuq[it // 2][:, :, (it % 2) * P : (it % 2 + 1) * P]

                    if len(blocks) > 1:
                        nc.gpsimd.dma_start(
                            x_sb[1],
                            seg["xT_r"][:, :, blocks[1][0] : blocks[1][0] + blocks[1][1]],
                        )
                    nc.gpsimd.dma_start(wal_sb, seg["wal"])
                    wd_sb = wpool.tile([P, IT, H], bf16, tag="wd", name="wd")
                    nc.gpsimd.dma_start(
                        wd_sb, seg["wd_r"][:, sup * IT : (sup + 1) * IT, :]
                    )
                    # Remaining x of wave 0, in need order.
                    for bi, (t0, tb) in enumerate(blocks):
                        if bi > 1:
                            nc.gpsimd.dma_start(
                                x_sb[bi], seg["xT_r"][:, :, t0 : t0 + tb]
                            )
                    nc.gpsimd.dma_start(segs["b"]["wal_sb"], segs["b"]["wal"])
                    for t_, src_ in prologue_b:
                        nc.gpsimd.dma_start(t_, src_)
                else:
                    wg_sb = wpool.tile([P, HO, IS], bf16, tag="wg", name="wg_sb")
                    nc.sync.dma_start(wg_sb, seg["wg_r"][:, :, i0 : i0 + IS])
                    wu_sb = wpool.tile([P, HO, IS], bf16, tag="wu", name="wu_sb")
                    nc.scalar.dma_start(wu_sb, seg["wu_r"][:, :, i0 : i0 + IS])

                    def wgt(it, wg_sb=wg_sb):
                        return wg_sb[:, :, it * P : (it + 1) * P]

                    def wut(it, wu_sb=wu_sb):
                        return wu_sb[:, :, it * P : (it + 1) * P]

                    wd_sb = wpool.tile([P, IT, H], bf16, tag="wd", name="wd")
                    nc.sync.dma_start(
                        wd_sb, seg["wd_r"][:, sup * IT : (sup + 1) * IT, :]
                    )

                def down_group(t0, h_sb, grp, seg=seg, sup=sup, wd_sb=wd_sb,
                               wal_sb=wal_sb, last_wave=(wi == len(waves) - 1)):
                    # One (token-subtile, output-half) group of the down
                    # projection, back to token-partition layout, scaled by
                    # the combine weight at PSUM eviction. Wave 0 writes
                    # f32; wave 1 writes bf16 via the HWDGE queues so the
                    # SWDGE queue is quiet long before the kernel tail.
                    tsub, hh = divmod(grp, HH)
                    col = t0 // P + tsub
                    r0 = t0 + tsub * P
                    ops = pout.tile([P, 512], f32, tag="o", name="o")
                    for it in range(IT):
                        nc.tensor.matmul(
                            ops,
                            lhsT=h_sb[:, it, tsub * P : (tsub + 1) * P],
                            rhs=wd_sb[:, it, hh * 512 : (hh + 1) * 512],
                            start=(it == 0),
                            stop=(it == IT - 1),
                        )
                    if sup == N_SUPER - 1:
                        oev2 = epool.tile([P, 512], bf16, tag="oev2", name="ov2")
                        nc.vector.tensor_scalar_mul(
                            oev2, ops, wal_sb[:, col : col + 1]
                        )
                        if last_wave:
                            eng = nc.sync if (col + hh) % 2 == 0 else nc.scalar
                        else:
                            eng = nc.gpsimd
                        eng.dma_start(
                            seg["out2"][r0 : r0 + P, hh * 512 : (hh + 1) * 512],
                            oev2,
                        )
                    else:
                        oev = epool.tile([P, 512], f32, tag="oev", name="oev")
                        nc.vector.tensor_scalar_mul(
                            oev, ops, wal_sb[:, col : col + 1]
                        )
                        nc.gpsimd.dma_start(
                            seg["out"][r0 : r0 + P, hh * 512 : (hh + 1) * 512],
                            oev,
                        )

                pending = None
                for bi, (t0, tb) in enumerate(blocks):
                    tsn = tb // P
                    # Expert FFN for this (i-chunk, token block):
                    # hT[i, t] = silu(Wg.T x)[i, t] * (Wu.T x)[i, t]
                    h_sb = hpool.tile([P, IT, TB], bf16, tag="h", name="h")[:, :, :tb]
                    dgn = tsn * HH
                    for it in range(IT):
                        gps = pgu.tile([P, TB], f32, tag="g", name="g")[:, :tb]
                        ups = pgu.tile([P, TB], f32, tag="u", name="u")[:, :tb]
                        for ho in range(HO):
                            nc.tensor.matmul(
                                gps,
                                lhsT=wgt(it)[:, ho, :],
                                rhs=x_sb[bi][:, ho, :],
                                start=(ho == 0),
                                stop=(ho == HO - 1),
                            )
                        for ho in range(HO):
                            nc.tensor.matmul(
                                ups,
                                lhsT=wut(it)[:, ho, :],
                                rhs=x_sb[bi][:, ho, :],
                                start=(ho == 0),
                                stop=(ho == HO - 1),
                            )
                        gs = epool.tile([P, TB], f32, tag="gs", name="gs")[:, :tb]
                        nc.scalar.activation(gs, gps, Act.Silu)
                        nc.vector.tensor_tensor(
                            h_sb[:, it, :], gs, ups, op=Alu.mult
                        )
                        if pending is not None:
                            p_t0, p_h, p_dgn, p_dg = pending
                            for grp in range(
                                it * p_dgn // IT, (it + 1) * p_dgn // IT
                            ):
                                p_dg(p_t0, p_h, grp)

                    if wi == 1:
                        for t_, src_ in b_load.get(bi, ()):
                            nc.gpsimd.dma_start(t_, src_)
                    pending = (t0, h_sb, dgn, down_group)
                if pending is not None:
                    p_t0, p_h, p_dgn, p_dg = pending
                    for grp in range(p_dgn):
                        p_dg(p_t0, p_h, grp)

    nc.compile()
    return nc


def _run_spmd(nc, in_maps, trace):
    from concourse import bass_utils

    if trace:
        try:
            res = bass_utils.run_bass_kernel_spmd(
                nc, in_maps, core_ids=list(range(E)), trace=True
            )
            if res.exec_time_ns is not None:
                print(f"HW exec time: {res.exec_time_ns} ns")
            return res
        except Exception as exc:  # fall back to an untraced run
            print(f"traced run failed ({exc!r}); retrying without trace")
    return bass_utils.run_bass_kernel_spmd(
        nc, in_maps, core_ids=list(range(E)), trace=False
    )


def prepare(hidden_states, gate_proj_w, gate_weights, up_weights, down_weights):
    """Host router + paired dispatch; returns (nc, in_maps, combine_fn)."""
    import ml_dtypes

    bf16 = ml_dtypes.bfloat16
    x = np.ascontiguousarray(hidden_states, dtype=np.float32)
    gpw = np.ascontiguousarray(gate_proj_w, dtype=np.float32)
    T = x.shape[0]

    # Router in fp64: logits -> softmax -> top-2 (stable ties like
    # jax.lax.top_k) -> renormalized combine weights.
    logits = x.astype(np.float64) @ gpw.astype(np.float64).T  # [T, E]
    pr = np.exp(logits - logits.max(axis=1, keepdims=True))
    pr /= pr.sum(axis=1, keepdims=True)
    top2 = np.argsort(-pr, axis=1, kind="stable")[:, :2]
    pv = np.take_along_axis(pr, top2, axis=1)
    wts = (pv / pv.sum(axis=1, keepdims=True)).astype(np.float32)  # [T, 2]

    idx = [np.nonzero((top2 == e).any(axis=1))[0] for e in range(E)]
    cnt = np.array([len(ix) for ix in idx])

    # Pair heaviest with lightest: segment A = 4 heaviest experts,
    # segment B = 4 lightest, pair rank k of A with rank -k of B.
    order = np.argsort(-cnt, kind="stable")
    pairs = [(int(order[k]), int(order[E - 1 - k])) for k in range(E // 2)]

    def pad128(n):
        return max(256, ((n + P - 1) // P) * P)

    TCA = pad128(max(cnt[a] for a, _ in pairs))
    TCB = pad128(max(cnt[b] for _, b in pairs))

    nc = build_moe(TCA, TCB)

    def seg_inputs(e, Tc, half):
        n_e = len(idx[e])
        xTe = np.zeros((H, Tc), dtype=bf16)
        if n_e:
            xTe[:, :n_e] = np.ascontiguousarray(x[idx[e]].T).astype(bf16)
        we = np.zeros((Tc,), dtype=np.float32)
        if n_e:
            we[:n_e] = np.where(
                top2[idx[e], 0] == e, wts[idx[e], 0], wts[idx[e], 1]
            )
        lo, hi = half * IH, (half + 1) * IH
        return {
            "x": xTe,
            "wg": np.ascontiguousarray(gate_weights[e][:, lo:hi]).astype(bf16),
            "wu": np.ascontiguousarray(up_weights[e][:, lo:hi]).astype(bf16),
            "wd": np.ascontiguousarray(down_weights[e][lo:hi, :]).astype(bf16),
            "wal": np.ascontiguousarray(we.reshape(Tc // P, P).T),
        }

    in_maps = []
    core_expert = []  # (expert_a, expert_b) per core
    for a, b in pairs:
        for half in range(2):
            sa = seg_inputs(a, TCA, half)
            sb = seg_inputs(b, TCB, half)
            in_maps.append(
                {
                    "xa": sa["x"], "wga": sa["wg"], "wua": sa["wu"],
                    "wda": sa["wd"], "wala": sa["wal"],
                    "xb": sb["x"], "wgb": sb["wg"], "wub": sb["wu"],
                    "wdb": sb["wd"], "walb": sb["wal"],
                }
            )
            core_expert.append((a, b))

    def combine(results):
        out = np.zeros((T, H), dtype=np.float32)
        for core, (a, b) in enumerate(core_expert):
            r = results[core]
            n_a = len(idx[a])
            if n_a:
                out[idx[a]] += (
                    r["outa"][:n_a] + r["out2a"][:n_a].astype(np.float32)
                )
            n_b = len(idx[b])
            if n_b:
                out[idx[b]] += (
                    r["outb"][:n_b] + r["out2b"][:n_b].astype(np.float32)
                )
        return out

    return nc, in_maps, combine


def kernel(hidden_states, gate_proj_w, gate_weights, up_weights, down_weights):
    trace = os.environ.get("MOE_TRACE", "0") == "1"
    nc, in_maps, combine = prepare(
        hidden_states, gate_proj_w, gate_weights, up_weights, down_weights
    )
    res = _run_spmd(nc, in_maps, trace)
    return combine(res.results)
